# revision 5
# baseline (speedup 1.0000x reference)
"""Trainium2 Bass kernel for nn_ContrastLoss (bidirectional NT-Xent-style loss).

v2 strategy (8 NeuronCores, SPMD), exploiting mp_sc = sc_mp.T:
  - Row-shard N=8192 over 8 cores; each core computes ONE [1024, 8192] block
    E = exp((z1n[rows] @ z2n.T)/tau) (the sc block). The mp block is its
    transpose, so instead of a second exp pass (the old baseline), the mp
    quantities are COLUMN sums of E:
      rs2[j] partial = sum_i E[i, j]        (mp denominators)
      mk2[j] partial = sum_i posT[i, j] E[i, j]
    computed via near-free PE matmuls (lhsT = data slice, rhs = ones[128,1],
    out [128,1] PSUM accumulated over row tiles), then shipped to the host
    which sums partials across cores and finishes the mp log-loss (tiny).
  - sc side stays on-chip: ACT exp w/ accum_out -> row sums; DVE
    scalar_tensor_tensor with the fp8 pos mask -> masked row sums; finale
    -log(...) -> per-core scalar.
  - Masks: pos rows in fp8e4 (0/1 exact; STT cost is dtype-independent),
    posT rows in bf16 (tensor_tensor needs all-2-byte operands for 2x mode).
  - Chunks are 1536 wide (5x1536 + 512 = 8192) so sim PSUM double-buffers in
    6 banks, leaving a bank for the column-sum strips.
"""

import os

import ml_dtypes
import numpy as np

import concourse.bass as bass
import concourse.mybir as mybir
import concourse.tile as tile
from concourse import bacc
from concourse.bass_utils import run_bass_kernel_spmd

F32 = mybir.dt.float32
BF16 = mybir.dt.bfloat16
FP8 = mybir.dt.float8e4
AF = mybir.ActivationFunctionType
ALU = mybir.AluOpType

N = 8192
HID = 64
M = 8              # cores
NM = N // M        # rows per core (1024)
NCAT = N + NM      # 9216 columns in the projected tensors (full | mine)
P = 128            # partitions
NT = NM // P       # row tiles per core (8)
CHW = [1536, 1536, 1536, 1536, 1536, 512]   # chunk widths (sum 8192)
CHO = [0, 1536, 3072, 4608, 6144, 7680]     # chunk col offsets
NCH = len(CHW)
MVW = 512          # matmul moving width (PSUM bank limit: 512 fp32 out)
PJW = 1024         # projection chunk width
NPJ = NCAT // PJW  # projection chunks (9)
NG = N // P        # column groups (64)
TAU = 0.8
LAMBDA = 0.5
EPS = 1e-8
NTOT = 48          # main-loop tiles per core
TT_GPS = int(os.environ.get("K_TTG", "30"))   # mp-TT tiles routed to GPSIMD
GPS_ELU = int(os.environ.get("K_ELUG", "1"))  # ELU tensor_scalar on GPSIMD


def _build_body(tc, ins, out_ap, outcs_ap):
    nc = tc.nc
    x1c, x2c, w1e, w2t, b2c, pos8, ptb = (
        ins["x1cat"], ins["x2cat"], ins["w1e"], ins["w2t"], ins["b2c"],
        ins["pos8"], ins["posTb"],
    )
    ds = bass.ds

    from contextlib import ExitStack

    with ExitStack() as ctx:
        const_pool = ctx.enter_context(tc.tile_pool(name="const", bufs=1))
        zn_pool = ctx.enter_context(tc.tile_pool(name="zn", bufs=1))
        strip_pool = ctx.enter_context(tc.tile_pool(name="strips", bufs=1))

        w1e_sb = const_pool.tile([65, HID], BF16, tag="w1e")
        nc.sync.dma_start(out=w1e_sb[:], in_=w1e[:])
        # W2.T duplicated at partition bases 0 and 64 (lhsT base must match rhs)
        w2x = const_pool.tile([P, HID], BF16, tag="w2x")
        nc.sync.dma_start(out=w2x[0:HID, :], in_=w2t[:])
        nc.sync.dma_start(out=w2x[HID:P, :], in_=w2t[:])
        b2c_sb = const_pool.tile([P, 1], F32, tag="b2c")
        nc.sync.dma_start(out=b2c_sb[:], in_=b2c[:])
        ones_col = const_pool.tile([P, 1], F32, tag="onesc")
        nc.vector.memset(ones_col[:], 1.0)
        ones_bf = const_pool.tile([P, 1], BF16, tag="onesb")
        nc.vector.memset(ones_bf[:], 1.0)
        # all-ones [128, 64] for column-sum broadcast matmuls (projection)
        ones_sq = const_pool.tile([P, HID], BF16, tag="onessq")
        nc.vector.memset(ones_sq[:], 1.0)

        # normalized projections, both views: rows 0:64 = z1n.T, 64:128 = z2n.T
        zn = zn_pool.tile([P, N], BF16, tag="zn")
        # mine block, swapped halves: rows 0:64 = z2n.T, 64:128 = z1n.T
        mine_sw = zn_pool.tile([P, NM], BF16, tag="minesw")

        # accumulator strips: per (row-tile, chunk) partials (sc stream only)
        rs1 = strip_pool.tile([P, NT * NCH], F32, tag="rs1")
        mk1 = strip_pool.tile([P, NT * NCH], F32, tag="mk1")
        if os.environ.get("K_NO_STT"):
            nc.vector.memset(mk1[:], 1.0)

        # ---------------- projection + normalization ----------------
        with ExitStack() as pctx:
            xc_pool = pctx.enter_context(tc.tile_pool(name="xc", bufs=1))
            helu_pool = pctx.enter_context(tc.tile_pool(name="helu", bufs=1))
            em_pool = pctx.enter_context(tc.tile_pool(name="em", bufs=4))
            zsq_pool = pctx.enter_context(tc.tile_pool(name="zsq", bufs=3))
            nrm_pool = pctx.enter_context(tc.tile_pool(name="nrm", bufs=3))
            pp = pctx.enter_context(tc.tile_pool(name="pp", bufs=4, space="PSUM"))

            x1_sb = xc_pool.tile([65, NCAT], BF16, tag="x1c")
            nc.sync.dma_start(out=x1_sb[:], in_=x1c[:])
            x2_sb = xc_pool.tile([65, NCAT], BF16, tag="x2c")
            nc.sync.dma_start(out=x2_sb[:], in_=x2c[:])

            helu = helu_pool.tile([P, NCAT], BF16, tag="helu")

            # ---- layer 1 + ELU:  helu = elu(W1 @ x.T + b1) for both views
            for c in range(NPJ):
                hp = pp.tile([P, PJW], F32, tag="pp")
                for q in range(PJW // MVW):
                    sl = ds(c * PJW + q * MVW, MVW)
                    qs = ds(q * MVW, MVW)
                    nc.tensor.matmul(hp[0:HID, qs], w1e_sb[:], x1_sb[:, sl],
                                     start=True, stop=True)
                    nc.tensor.matmul(hp[HID:P, qs], w1e_sb[:], x2_sb[:, sl],
                                     start=True, stop=True)
                sl = ds(c * PJW, PJW)
                e_t = em_pool.tile([P, PJW], F32, tag="em")
                nc.scalar.activation(e_t[:], hp[:], AF.Exp)
                m_t = em_pool.tile([P, PJW], F32, tag="em")
                ts_eng = nc.gpsimd if GPS_ELU else nc.vector
                ts_eng.tensor_scalar(m_t[:], e_t[:], 1.0, -1.0, op0=ALU.min,
                                     op1=ALU.add)
                # elu(h) = max(h, min(exp(h),1)-1)
                nc.vector.tensor_tensor(helu[:, sl], hp[:], m_t[:], op=ALU.max)

            # ---- layer 2 + normalize, fully chunk-local:
            # zp = W2 @ helu (+swap halves for the mine chunk); ssq broadcast
            # via ones-matmul; inv = rsqrt(ssq); zn = (zp+b2)*inv in one STT
            for c in range(NPJ):
                is_mine = c * PJW >= N
                zp = pp.tile([P, PJW], F32, tag="pp")
                sb = pp.tile([P, PJW], F32, tag="pp")
                if not is_mine:
                    d1, d2 = slice(0, HID), slice(HID, P)
                else:  # swap output halves for the mine block
                    d1, d2 = slice(HID, P), slice(0, HID)
                for q in range(PJW // MVW):
                    sl = ds(c * PJW + q * MVW, MVW)
                    qs = ds(q * MVW, MVW)
                    nc.tensor.matmul(zp[d1, qs], w2x[0:HID, :],
                                     helu[0:HID, sl], start=True, stop=True)
                    nc.tensor.matmul(zp[d2, qs], w2x[HID:P, :],
                                     helu[HID:P, sl], start=True, stop=True)
                zq = zsq_pool.tile([P, PJW], BF16, tag="zsq")
                nc.scalar.activation(zq[:], zp[:], AF.Square)
                for q in range(PJW // MVW):
                    qs = ds(q * MVW, MVW)
                    nc.tensor.matmul(sb[0:HID, qs], ones_sq[0:HID, :],
                                     zq[0:HID, qs], start=True, stop=True)
                    nc.tensor.matmul(sb[HID:P, qs], ones_sq[HID:P, :],
                                     zq[HID:P, qs], start=True, stop=True)
                invb = nrm_pool.tile([P, PJW], F32, tag="invb")
                nc.scalar.activation(invb[:], sb[:], AF.Abs_reciprocal_sqrt)
                dst = (zn[:, ds(c * PJW, PJW)] if not is_mine
                       else mine_sw[:, ds(c * PJW - N, PJW)])
                nc.vector.scalar_tensor_tensor(
                    out=dst, in0=zp[:], scalar=b2c_sb[:], in1=invb[:],
                    op0=ALU.add, op1=ALU.mult)

        # ---------------- main similarity loop (single E block) ----------
        with ExitStack() as mctx:
            pos_pool = mctx.enter_context(tc.tile_pool(name="pos", bufs=4))
            post_pool = mctx.enter_context(tc.tile_pool(name="post", bufs=4))
            e_pool = mctx.enter_context(tc.tile_pool(name="et", bufs=4))
            s1_pool = mctx.enter_context(tc.tile_pool(name="s1", bufs=3))
            s2_pool = mctx.enter_context(tc.tile_pool(name="s2", bufs=3))
            pm = mctx.enter_context(tc.tile_pool(name="pm", bufs=2, space="PSUM"))
            csp = mctx.enter_context(tc.tile_pool(name="csp", bufs=1, space="PSUM"))

            # column-sum strips: [:, 0:64] = colsum(E), [:, 64:128] = masked
            cs = csp.tile([P, 2 * NG], F32, tag="cs")

            lhsT = mine_sw[HID:P, :]   # z1n mine rows
            rh = zn[HID:P, :]          # z2n full
            for t in range(NT):
                tsl = ds(t * P, P)
                for ch in range(NCH):
                    W, C = CHW[ch], CHO[ch]
                    col = ds(t * NCH + ch, 1)
                    pt = pos_pool.tile([P, 1536], FP8, tag="pos")
                    qt = post_pool.tile([P, 1536], BF16, tag="post")
                    if not os.environ.get("K_NO_DMA"):
                        nc.sync.dma_start(out=pt[:, 0:W],
                                          in_=pos8[tsl, ds(C, W)])
                        nc.sync.dma_start(out=qt[:, 0:W],
                                          in_=ptb[tsl, ds(C, W)])
                    elif t == 0 and ch == 0:
                        nc.vector.memset(pt[:], 1.0)
                        nc.vector.memset(qt[:], 1.0)
                    ps = pm.tile([P, 1536], F32, tag="pm")
                    for q in range(W // MVW):
                        nc.tensor.matmul(
                            ps[:, ds(q * MVW, MVW)], lhsT[:, tsl],
                            rh[:, ds(C + q * MVW, MVW)],
                            start=True, stop=True)
                    e_t = e_pool.tile([P, 1536], BF16, tag="et")
                    if not os.environ.get("K_NO_ACT"):
                        nc.scalar.activation(e_t[:, 0:W], ps[:, 0:W], AF.Exp,
                                             scale=float(1.0 / TAU),
                                             accum_out=rs1[:, col])
                    elif t == 0 and ch == 0:
                        nc.vector.memset(e_t[:], 1.0)
                        nc.vector.memset(rs1[:], 1.0)
                    if not os.environ.get("K_NO_STT"):
                        scr1 = s1_pool.tile([P, 1536], BF16, tag="scr1")
                        # masked sc row-sum: DVE only (Pool cannot run STT)
                        nc.vector.scalar_tensor_tensor(
                            out=scr1[:, 0:W], in0=e_t[:, 0:W], scalar=1.0,
                            in1=pt[:, 0:W], op0=ALU.mult, op1=ALU.mult,
                            accum_out=mk1[:, col])
                    scr2 = s2_pool.tile([P, 1536], BF16, tag="scr2")
                    if not os.environ.get("K_NO_TT"):
                        # mp-mask multiply: split DVE / GPSIMD to balance
                        eng = (nc.gpsimd
                               if (t * NCH + ch) % NTOT < TT_GPS else nc.vector)
                        eng.tensor_tensor(scr2[:, 0:W], e_t[:, 0:W],
                                          qt[:, 0:W], op=ALU.mult)
                    # near-free column sums: lhsT = data slice, rhs = ones
                    if not os.environ.get("K_NO_CS"):
                        for b in range(W // P):
                            g = C // P + b
                            bsl = ds(b * P, P)
                            nc.tensor.matmul(cs[:, ds(g, 1)], e_t[:, bsl],
                                             ones_bf[:], start=(t == 0),
                                             stop=(t == NT - 1))
                            if not os.environ.get("K_NO_TT"):
                                nc.tensor.matmul(cs[:, ds(NG + g, 1)],
                                                 scr2[:, bsl], ones_bf[:],
                                                 start=(t == 0),
                                                 stop=(t == NT - 1))

            cs_sb = strip_pool.tile([P, 2 * NG], F32, tag="cs_sb")
            if os.environ.get("K_NO_CS") or os.environ.get("K_NO_TT"):
                nc.vector.memset(cs_sb[:], 1.0)
            else:
                nc.vector.tensor_copy(cs_sb[:], cs[:])
            nc.sync.dma_start(out=outcs_ap[:], in_=cs_sb[:])

        # ---------------- finale (sc stream only) ----------------
        with ExitStack() as fctx:
            fin_pool = fctx.enter_context(tc.tile_pool(name="fin", bufs=1))
            pf = fctx.enter_context(tc.tile_pool(name="pf", bufs=1, space="PSUM"))

            lnin = fin_pool.tile([P, NT], F32, tag="lnin")
            lnout = fin_pool.tile([P, NT], F32, tag="lnout")
            rsf = fin_pool.tile([P, NT], F32, tag="rsf")
            mkf = fin_pool.tile([P, NT], F32, tag="mkf")
            nc.vector.tensor_reduce(
                rsf[:], rs1[:].rearrange("p (t k) -> p t k", k=NCH),
                axis=mybir.AxisListType.X, op=ALU.add)
            nc.vector.tensor_reduce(
                mkf[:], mk1[:].rearrange("p (t k) -> p t k", k=NCH),
                axis=mybir.AxisListType.X, op=ALU.add)
            den = fin_pool.tile([P, NT], F32, tag="den")
            nc.vector.tensor_scalar(den[:], rsf[:], float(EPS), None,
                                    op0=ALU.add)
            rec = fin_pool.tile([P, NT], F32, tag="rec")
            nc.vector.reciprocal(rec[:], den[:])
            nc.vector.tensor_tensor(lnin[:], mkf[:], rec[:], op=ALU.mult)
            lnacc = fin_pool.tile([P, 1], F32, tag="lnacc")
            epsb = fin_pool.tile([P, 1], F32, tag="epsb")
            nc.vector.memset(epsb[:], float(EPS))
            nc.scalar.activation(lnout[:], lnin[:], AF.Ln, bias=epsb[:],
                                 accum_out=lnacc[:])
            ps1 = pf.tile([1, 1], F32, tag="pf")
            nc.tensor.matmul(ps1[:], ones_col[:], lnacc[:], start=True,
                             stop=True)
            res = fin_pool.tile([1, 1], F32, tag="res")
            # sum over core rows, x(-lambda/N): host adds mp part
            nc.scalar.activation(res[:], ps1[:], AF.Copy,
                                 scale=float(-LAMBDA / N))
            nc.sync.dma_start(out=out_ap[:], in_=res[:])


_CACHE = {}


def _build_program():
    if "nc" in _CACHE:
        return _CACHE["nc"]
    nc = bacc.Bacc("TRN2", target_bir_lowering=False, debug=False,
                   num_devices=M)
    ins = {
        "x1cat": nc.dram_tensor("x1cat", [65, NCAT], BF16, kind="ExternalInput").ap(),
        "x2cat": nc.dram_tensor("x2cat", [65, NCAT], BF16, kind="ExternalInput").ap(),
        "w1e": nc.dram_tensor("w1e", [65, HID], BF16, kind="ExternalInput").ap(),
        "w2t": nc.dram_tensor("w2t", [HID, HID], BF16, kind="ExternalInput").ap(),
        "b2c": nc.dram_tensor("b2c", [P, 1], F32, kind="ExternalInput").ap(),
        "pos8": nc.dram_tensor("pos8", [NM, N], FP8, kind="ExternalInput").ap(),
        "posTb": nc.dram_tensor("posTb", [NM, N], BF16, kind="ExternalInput").ap(),
    }
    out_ap = nc.dram_tensor("out", [1, 1], F32, kind="ExternalOutput").ap()
    outcs_ap = nc.dram_tensor("outcs", [P, 2 * NG], F32,
                              kind="ExternalOutput").ap()
    with tile.TileContext(nc) as tc:
        _build_body(tc, ins, out_ap, outcs_ap)
    nc.compile()
    _CACHE["nc"] = nc
    return nc


def _host_prep(x1, x2, W1, b1, W2, b2, positive_matrix):
    f32 = np.float32
    bf = ml_dtypes.bfloat16
    fp8 = ml_dtypes.float8_e4m3fn
    x1t = np.asarray(x1, f32).T
    x2t = np.asarray(x2, f32).T
    ones = np.ones((1, N), f32)
    w1e = np.ascontiguousarray(np.concatenate(
        [np.asarray(W1, f32).T, np.asarray(b1, f32)[None, :]], axis=0
    ).astype(bf))
    base1 = np.concatenate([x1t, ones], axis=0).astype(bf)   # [65, N]
    base2 = np.concatenate([x2t, ones], axis=0).astype(bf)
    w2t = np.ascontiguousarray(np.asarray(W2, f32).T.astype(bf))
    b2c = np.concatenate([np.asarray(b2, f32)] * 2)[:, None].copy()
    pos = np.ascontiguousarray(positive_matrix, dtype=f32)
    in_maps = []
    for c in range(M):
        rc = slice(c * NM, (c + 1) * NM)
        in_maps.append({
            "x1cat": np.ascontiguousarray(
                np.concatenate([base1, base1[:, rc]], axis=1)),
            "x2cat": np.ascontiguousarray(
                np.concatenate([base2, base2[:, rc]], axis=1)),
            "w1e": w1e,
            "w2t": w2t,
            "b2c": b2c,
            "pos8": np.ascontiguousarray(pos[rc]).astype(fp8),
            "posTb": np.ascontiguousarray(pos[:, rc].T).astype(bf),
        })
    return in_maps


def run_on_hw(in_maps, trace=False, **kw):
    nc = _build_program()
    return run_bass_kernel_spmd(nc, in_maps, list(range(M)), trace=trace, **kw)


def kernel(x1, x2, W1, b1, W2, b2, positive_matrix):
    in_maps = _host_prep(x1, x2, W1, b1, W2, b2, positive_matrix)
    res = run_on_hw(in_maps)
    sc_part = np.float64(0.0)
    rs2 = np.zeros(N, dtype=np.float64)
    mk2 = np.zeros(N, dtype=np.float64)
    for c in range(M):
        sc_part += np.float64(res.results[c]["out"][0, 0])
        csm = np.asarray(res.results[c]["outcs"], dtype=np.float64)
        # column (128*g + p) -> csm[p, g]
        rs2 += csm[:, 0:NG].T.reshape(N)
        mk2 += csm[:, NG:2 * NG].T.reshape(N)
    mp_loss = -np.mean(np.log(mk2 / (rs2 + EPS) + EPS))
    total = sc_part + (1.0 - LAMBDA) * mp_loss
    return np.float32(total)



# revision 40
# speedup vs baseline: 1.2688x; 1.2688x over previous
"""Trainium2 Bass kernel for nn_ContrastLoss (bidirectional NT-Xent-style loss).

v2 strategy (8 NeuronCores, SPMD), exploiting mp_sc = sc_mp.T:
  - Row-shard N=8192 over 8 cores; each core computes ONE [1024, 8192] block
    E = exp((z1n[rows] @ z2n.T)/tau) (the sc block). The mp block is its
    transpose, so instead of a second exp pass (the old baseline), the mp
    quantities are COLUMN sums of E:
      rs2[j] partial = sum_i E[i, j]        (mp denominators)
      mk2[j] partial = sum_i posT[i, j] E[i, j]
    computed via near-free PE matmuls (lhsT = data slice, rhs = ones[128,1],
    out [128,1] PSUM accumulated over row tiles), then shipped to the host
    which sums partials across cores and finishes the mp log-loss (tiny).
  - sc side stays on-chip: ACT exp w/ accum_out -> row sums; DVE
    scalar_tensor_tensor with the fp8 pos mask -> masked row sums; finale
    -log(...) -> per-core scalar.
  - Masks: pos rows in fp8e4 (0/1 exact; STT cost is dtype-independent),
    posT rows in bf16 (tensor_tensor needs all-2-byte operands for 2x mode).
  - Chunks are 1536 wide (5x1536 + 512 = 8192) so sim PSUM double-buffers in
    6 banks, leaving a bank for the column-sum strips.
"""

import os

import ml_dtypes
import numpy as np

import concourse.bass as bass
import concourse.mybir as mybir
import concourse.tile as tile
from concourse import bacc
from concourse.bass_utils import run_bass_kernel_spmd

F32 = mybir.dt.float32
BF16 = mybir.dt.bfloat16
FP8 = mybir.dt.float8e4
AF = mybir.ActivationFunctionType
ALU = mybir.AluOpType

N = 8192
HID = 64
M = 8              # cores
NM = N // M        # rows per core (1024)
NCAT = N + NM      # 9216 columns in the projected tensors (full | mine)
P = 128            # partitions
NT = NM // P       # row tiles per core (8)
CHW = [1536, 1536, 1536, 1536, 1536, 512]   # main chunk widths (sum 8192)
CHO = [0, 1536, 3072, 4608, 6144, 7680]     # main chunk col offsets
NCH = len(CHW)
MVW = 512          # matmul moving width (PSUM bank limit: 512 fp32 out)
PJW = int(os.environ.get("K_PJW", "512"))  # projection chunk width
NPJ = NCAT // PJW  # projection chunks (9); chunk 8 = the "mine" block
ZNW = [3072, 3072, 2048]   # zn split: 1024-writable, 1536-readable
ZNO = [0, 3072, 6144]
NG = N // P        # column groups (64)
TAU = 0.8
LAMBDA = 0.5
EPS = 1e-8
TT_GPS = int(os.environ.get("K_TTG", "7"))    # mp-TT tiles per 12 on GPSIMD
GPS_ELU = int(os.environ.get("K_ELUG", "0"))  # ELU tensor_scalar on GPSIMD
CS_DEFER = int(os.environ.get("K_CSD", "2"))  # colsum emission deferral depth
PJ_LAG = int(os.environ.get("K_PJL", "9"))    # projection L2 lag behind L1


def _build_body(tc, ins, out_ap, outcs_ap):
    nc = tc.nc
    x1c, x2c, w1e, w2t, b2c, pos8, ptb = (
        ins["x1cat"], ins["x2cat"], ins["w1e"], ins["w2t"], ins["b2c"],
        ins["pos8"], ins["posTb"],
    )
    ds = bass.ds

    from contextlib import ExitStack

    with ExitStack() as ctx:
        const_pool = ctx.enter_context(tc.tile_pool(name="const", bufs=1))
        zn_pool = ctx.enter_context(tc.tile_pool(name="zn", bufs=1))
        strip_pool = ctx.enter_context(tc.tile_pool(name="strips", bufs=1))

        w1e_sb = const_pool.tile([65, HID], BF16, tag="w1e")
        nc.sync.dma_start(out=w1e_sb[:], in_=w1e[:])
        # W2.T duplicated at partition bases 0 and 64 (lhsT base must match rhs)
        w2x = const_pool.tile([P, HID], BF16, tag="w2x")
        nc.sync.dma_start(out=w2x[0:HID, :], in_=w2t[:])
        nc.sync.dma_start(out=w2x[HID:P, :], in_=w2t[:])
        b2c_sb = const_pool.tile([P, 1], F32, tag="b2c")
        nc.sync.dma_start(out=b2c_sb[:], in_=b2c[:])
        ones_col = const_pool.tile([P, 1], F32, tag="onesc")
        nc.vector.memset(ones_col[:], 1.0)
        ones_bf = const_pool.tile([P, 1], BF16, tag="onesb")
        nc.vector.memset(ones_bf[:], 1.0)
        # all-ones [128, 64] for column-sum broadcast matmuls (projection)
        ones_sq = const_pool.tile([P, HID], BF16, tag="onessq")
        nc.vector.memset(ones_sq[:], 1.0)

        # normalized projections, both views: rows 0:64 = z1n.T, 64:128 = z2n.T
        # split in 3 tiles so main-loop tiles depend on a third of the
        # projection, not all of it (3072 boundaries suit both 1024-wide
        # projection writes and 1536-wide main-loop reads)
        zn_t = [zn_pool.tile([P, w], BF16, tag=f"zn{i}", name=f"zn{i}")
                for i, w in enumerate(ZNW)]
        # mine block, swapped halves: rows 0:64 = z2n.T, 64:128 = z1n.T
        mine_sw = zn_pool.tile([P, NM], BF16, tag="minesw")

        # accumulator strips: per (row-tile, chunk) partials (sc stream only)
        rs1 = strip_pool.tile([P, NT * NCH], F32, tag="rs1")
        mk1 = strip_pool.tile([P, NT * NCH], F32, tag="mk1")
        if os.environ.get("K_NO_STT"):
            nc.vector.memset(mk1[:], 1.0)

        # ---------------- projection + normalization ----------------
        # Software-pipelined: per-chunk tiles for helu/zn; layer 2 of chunk
        # c-2 is emitted right after layer 1 of chunk c, so L1's DVE work
        # (ELU) overlaps L2's ACT work (square/rsqrt) across chunks. The
        # "mine" chunk (8) comes first so the main loop's lhsT is ready early.
        with ExitStack() as pctx:
            xc_pool = pctx.enter_context(tc.tile_pool(name="xc", bufs=1))
            helu_pool = pctx.enter_context(tc.tile_pool(name="helu", bufs=1))
            em_pool = pctx.enter_context(tc.tile_pool(name="em", bufs=6))
            zsq_pool = pctx.enter_context(tc.tile_pool(name="zsq", bufs=4))
            nrm_pool = pctx.enter_context(tc.tile_pool(name="nrm", bufs=4))

            x1_t = [xc_pool.tile([65, PJW], BF16, tag=f"x1c{c}",
                                 name=f"x1c{c}") for c in range(NPJ)]
            x2_t = [xc_pool.tile([65, PJW], BF16, tag=f"x2c{c}",
                                 name=f"x2c{c}") for c in range(NPJ)]

            helu_t = [helu_pool.tile([P, PJW], BF16, tag=f"helu{c}",
                                     name=f"helu{c}") for c in range(NPJ)]

            pjb = 2 if PJW > 512 else 4
            pp = pctx.enter_context(tc.tile_pool(name="pp", bufs=pjb, space="PSUM"))
            zpp = pctx.enter_context(tc.tile_pool(name="zpp", bufs=pjb, space="PSUM"))

            def proj_l1(c):
                # layer 1 + ELU: helu_t[c] = elu(W1 @ x.T + b1), both views
                hp = pp.tile([P, PJW], F32, tag="pp")
                for q in range(PJW // MVW):
                    sl = ds(q * MVW, MVW)
                    qs = ds(q * MVW, MVW)
                    nc.tensor.matmul(hp[0:HID, qs], w1e_sb[:], x1_t[c][:, sl],
                                     start=True, stop=True)
                    nc.tensor.matmul(hp[HID:P, qs], w1e_sb[:], x2_t[c][:, sl],
                                     start=True, stop=True)
                e_t = em_pool.tile([P, PJW], F32, tag="em")
                nc.scalar.activation(e_t[:], hp[:], AF.Exp)
                m_t = em_pool.tile([P, PJW], F32, tag="em")
                ts_eng = nc.gpsimd if GPS_ELU else nc.vector
                ts_eng.tensor_scalar(m_t[:], e_t[:], 1.0, -1.0, op0=ALU.min,
                                     op1=ALU.add)
                # elu(h) = max(h, min(exp(h),1)-1)
                nc.vector.tensor_tensor(helu_t[c][:], hp[:], m_t[:],
                                        op=ALU.max)

            def proj_l2(c):
                # layer 2 + normalize, chunk-local: zp = W2 @ helu (+swap
                # halves for mine); ssq via ones-matmul; inv = rsqrt(ssq);
                # zn = (zp+b2)*inv fused in one STT
                is_mine = c * PJW >= N
                zp = zpp.tile([P, PJW], F32, tag="zpp")
                sb = pp.tile([P, PJW], F32, tag="pp")
                if not is_mine:
                    d1, d2 = slice(0, HID), slice(HID, P)
                else:  # swap output halves for the mine block
                    d1, d2 = slice(HID, P), slice(0, HID)
                for q in range(PJW // MVW):
                    qs = ds(q * MVW, MVW)
                    nc.tensor.matmul(zp[d1, qs], w2x[0:HID, :],
                                     helu_t[c][0:HID, qs], start=True,
                                     stop=True)
                    nc.tensor.matmul(zp[d2, qs], w2x[HID:P, :],
                                     helu_t[c][HID:P, qs], start=True,
                                     stop=True)
                zq = zsq_pool.tile([P, PJW], BF16, tag="zsq")
                # alternate square between ACT and DVE: ACT paces this phase
                if c % 2 == 0 or int(os.environ.get("K_SQA", "1")):
                    nc.scalar.activation(zq[:], zp[:], AF.Square)
                else:
                    nc.vector.scalar_tensor_tensor(
                        out=zq[:], in0=zp[:], scalar=1.0, in1=zp[:],
                        op0=ALU.mult, op1=ALU.mult)
                for q in range(PJW // MVW):
                    qs = ds(q * MVW, MVW)
                    nc.tensor.matmul(sb[0:HID, qs], ones_sq[0:HID, :],
                                     zq[0:HID, qs], start=True, stop=True)
                    nc.tensor.matmul(sb[HID:P, qs], ones_sq[HID:P, :],
                                     zq[HID:P, qs], start=True, stop=True)
                invb = nrm_pool.tile([P, PJW], F32, tag="invb")
                nc.scalar.activation(invb[:], sb[:], AF.Abs_reciprocal_sqrt)
                if is_mine:
                    dst = mine_sw[:, ds(c * PJW - N, PJW)]
                else:
                    zi = (c * PJW) // 3072
                    dst = zn_t[zi][:, ds(c * PJW - ZNO[zi], PJW)]
                nc.vector.scalar_tensor_tensor(
                    out=dst, in0=zp[:], scalar=b2c_sb[:], in1=invb[:],
                    op0=ALU.add, op1=ALU.mult)

            nmine = NM // PJW
            order = list(range(NPJ - nmine, NPJ)) + list(range(NPJ - nmine))
            for c in order:  # input slices, in consumption order
                nc.sync.dma_start(out=x1_t[c][:], in_=x1c[:, ds(c * PJW, PJW)])
                nc.sync.dma_start(out=x2_t[c][:], in_=x2c[:, ds(c * PJW, PJW)])
            for i, c in enumerate(order):
                proj_l1(c)
                if i >= PJ_LAG:
                    proj_l2(order[i - PJ_LAG])
            for c in order[len(order) - PJ_LAG:]:
                proj_l2(c)

        # ---------------- main similarity loop (single E block) ----------
        with ExitStack() as mctx:
            pos_pool = mctx.enter_context(tc.tile_pool(name="pos", bufs=4))
            post_pool = mctx.enter_context(tc.tile_pool(name="post", bufs=4))
            e_pool = mctx.enter_context(tc.tile_pool(name="et", bufs=int(os.environ.get("K_EB", "8"))))
            s1_pool = mctx.enter_context(tc.tile_pool(name="s1", bufs=3))
            s2_pool = mctx.enter_context(tc.tile_pool(name="s2", bufs=6))
            pm = mctx.enter_context(tc.tile_pool(name="pm", bufs=2, space="PSUM"))
            csp = mctx.enter_context(tc.tile_pool(name="csp", bufs=1, space="PSUM"))

            # column-sum strips: [:, 0:64] = colsum(E), [:, 64:128] = masked
            cs = csp.tile([P, 2 * NG], F32, tag="cs")

            lhsT = mine_sw[HID:P, :]   # z1n mine rows

            def emit_colsums(item):
                # near-free column sums: lhsT = data slice, rhs = ones.
                # Emitted DEFERRED (a couple of tiles late) so the PE never
                # stalls waiting on the DVE/GPSIMD mask products: its in-order
                # stream would otherwise serialize every tile.
                it, ich, ie_t, iscr2 = item
                for b in range(CHW[ich] // P):
                    g = CHO[ich] // P + b
                    bsl = ds(b * P, P)
                    nc.tensor.matmul(cs[:, ds(g, 1)], ie_t[:, bsl],
                                     ones_bf[:], start=(it == 0),
                                     stop=(it == NT - 1))
                    if not os.environ.get("K_NO_TT"):
                        nc.tensor.matmul(cs[:, ds(NG + g, 1)],
                                         iscr2[:, bsl], ones_bf[:],
                                         start=(it == 0),
                                         stop=(it == NT - 1))

            pending = []
            for t in range(NT):
                tsl = ds(t * P, P)
                for ch in range(NCH):
                    W, C = CHW[ch], CHO[ch]
                    col = ds(t * NCH + ch, 1)
                    pt = pos_pool.tile([P, 1536], FP8, tag="pos")
                    qt = post_pool.tile([P, 1536], BF16, tag="post")
                    if not os.environ.get("K_NO_DMA"):
                        nc.sync.dma_start(out=pt[:, 0:W],
                                          in_=pos8[tsl, ds(C, W)])
                        nc.sync.dma_start(out=qt[:, 0:W],
                                          in_=ptb[tsl, ds(C, W)])
                    elif t == 0 and ch == 0:
                        nc.vector.memset(pt[:], 1.0)
                        nc.vector.memset(qt[:], 1.0)
                    ps = pm.tile([P, 1536], F32, tag="pm")
                    zi = C // 3072
                    for q in range(W // MVW):
                        nc.tensor.matmul(
                            ps[:, ds(q * MVW, MVW)], lhsT[:, tsl],
                            zn_t[zi][HID:P, ds(C - ZNO[zi] + q * MVW, MVW)],
                            start=True, stop=True)
                    e_t = e_pool.tile([P, 1536], BF16, tag="et")
                    if not os.environ.get("K_NO_ACT"):
                        nc.scalar.activation(e_t[:, 0:W], ps[:, 0:W], AF.Exp,
                                             scale=float(1.0 / TAU),
                                             accum_out=rs1[:, col])
                    elif t == 0 and ch == 0:
                        nc.vector.memset(e_t[:], 1.0)
                        nc.vector.memset(rs1[:], 1.0)
                    if not os.environ.get("K_NO_STT"):
                        scr1 = s1_pool.tile([P, 1536], BF16, tag="scr1")
                        # masked sc row-sum: DVE only (Pool cannot run STT)
                        nc.vector.scalar_tensor_tensor(
                            out=scr1[:, 0:W], in0=e_t[:, 0:W], scalar=1.0,
                            in1=pt[:, 0:W], op0=ALU.mult, op1=ALU.mult,
                            accum_out=mk1[:, col])
                    scr2 = s2_pool.tile([P, 1536], BF16, tag="scr2")
                    if not os.environ.get("K_NO_TT"):
                        # mp-mask multiply: split DVE / GPSIMD to balance,
                        # interleaved so neither engine gets a long solo run
                        idx = t * NCH + ch
                        eng = (nc.gpsimd
                               if (idx * TT_GPS) % 12 < TT_GPS or idx >= int(os.environ.get("K_TAILG", "42"))
                               else nc.vector)
                        eng.tensor_tensor(scr2[:, 0:W], e_t[:, 0:W],
                                          qt[:, 0:W], op=ALU.mult)
                    if not os.environ.get("K_NO_CS"):
                        pending.append((t, ch, e_t, scr2))
                        if len(pending) > CS_DEFER:
                            emit_colsums(pending.pop(0))
            for item in pending:
                emit_colsums(item)

            cs_sb = strip_pool.tile([P, 2 * NG], F32, tag="cs_sb")
            if os.environ.get("K_NO_CS") or os.environ.get("K_NO_TT"):
                nc.vector.memset(cs_sb[:], 1.0)
            else:
                nc.vector.tensor_copy(cs_sb[:], cs[:])
            nc.sync.dma_start(out=outcs_ap[:], in_=cs_sb[:])

        # ---------------- finale: ship raw strips; host does the tiny
        # log-loss math (it already does the mp half) ----------------
        nc.sync.dma_start(out=out_ap[:, 0:NT * NCH], in_=rs1[:])
        nc.sync.dma_start(out=out_ap[:, NT * NCH:2 * NT * NCH], in_=mk1[:])


_CACHE = {}


def _build_program():
    if "nc" in _CACHE:
        return _CACHE["nc"]
    nc = bacc.Bacc("TRN2", target_bir_lowering=False, debug=False,
                   num_devices=M)
    ins = {
        "x1cat": nc.dram_tensor("x1cat", [65, NCAT], BF16, kind="ExternalInput").ap(),
        "x2cat": nc.dram_tensor("x2cat", [65, NCAT], BF16, kind="ExternalInput").ap(),
        "w1e": nc.dram_tensor("w1e", [65, HID], BF16, kind="ExternalInput").ap(),
        "w2t": nc.dram_tensor("w2t", [HID, HID], BF16, kind="ExternalInput").ap(),
        "b2c": nc.dram_tensor("b2c", [P, 1], F32, kind="ExternalInput").ap(),
        "pos8": nc.dram_tensor("pos8", [NM, N], FP8, kind="ExternalInput").ap(),
        "posTb": nc.dram_tensor("posTb", [NM, N], BF16, kind="ExternalInput").ap(),
    }
    out_ap = nc.dram_tensor("out", [P, 2 * NT * NCH], F32,
                            kind="ExternalOutput").ap()
    outcs_ap = nc.dram_tensor("outcs", [P, 2 * NG], F32,
                              kind="ExternalOutput").ap()
    with tile.TileContext(nc) as tc:
        _build_body(tc, ins, out_ap, outcs_ap)
    nc.compile()
    _CACHE["nc"] = nc
    return nc


def _host_prep(x1, x2, W1, b1, W2, b2, positive_matrix):
    f32 = np.float32
    bf = ml_dtypes.bfloat16
    fp8 = ml_dtypes.float8_e4m3fn
    x1t = np.asarray(x1, f32).T
    x2t = np.asarray(x2, f32).T
    ones = np.ones((1, N), f32)
    w1e = np.ascontiguousarray(np.concatenate(
        [np.asarray(W1, f32).T, np.asarray(b1, f32)[None, :]], axis=0
    ).astype(bf))
    base1 = np.concatenate([x1t, ones], axis=0).astype(bf)   # [65, N]
    base2 = np.concatenate([x2t, ones], axis=0).astype(bf)
    w2t = np.ascontiguousarray(np.asarray(W2, f32).T.astype(bf))
    b2c = np.concatenate([np.asarray(b2, f32)] * 2)[:, None].copy()
    pos = np.ascontiguousarray(positive_matrix, dtype=f32)
    in_maps = []
    for c in range(M):
        rc = slice(c * NM, (c + 1) * NM)
        in_maps.append({
            "x1cat": np.ascontiguousarray(
                np.concatenate([base1, base1[:, rc]], axis=1)),
            "x2cat": np.ascontiguousarray(
                np.concatenate([base2, base2[:, rc]], axis=1)),
            "w1e": w1e,
            "w2t": w2t,
            "b2c": b2c,
            "pos8": np.ascontiguousarray(pos[rc]).astype(fp8),
            "posTb": np.ascontiguousarray(pos[:, rc].T).astype(bf),
        })
    return in_maps


def run_on_hw(in_maps, trace=False, **kw):
    nc = _build_program()
    return run_bass_kernel_spmd(nc, in_maps, list(range(M)), trace=trace, **kw)


def kernel(x1, x2, W1, b1, W2, b2, positive_matrix):
    in_maps = _host_prep(x1, x2, W1, b1, W2, b2, positive_matrix)
    res = run_on_hw(in_maps)
    sc_sum = np.float64(0.0)
    rs2 = np.zeros(N, dtype=np.float64)
    mk2 = np.zeros(N, dtype=np.float64)
    for c in range(M):
        st = np.asarray(res.results[c]["out"], dtype=np.float64)
        rs1 = st[:, 0:NT * NCH].reshape(P, NT, NCH).sum(axis=2)   # [128, 8]
        mk1 = st[:, NT * NCH:].reshape(P, NT, NCH).sum(axis=2)
        sc_sum += np.log(mk1 / (rs1 + EPS) + EPS).sum()
        csm = np.asarray(res.results[c]["outcs"], dtype=np.float64)
        # column (128*g + p) -> csm[p, g]
        rs2 += csm[:, 0:NG].T.reshape(N)
        mk2 += csm[:, NG:2 * NG].T.reshape(N)
    sc_loss = -sc_sum / N
    mp_loss = -np.mean(np.log(mk2 / (rs2 + EPS) + EPS))
    total = LAMBDA * sc_loss + (1.0 - LAMBDA) * mp_loss
    return np.float32(total)

